# revision 5
# baseline (speedup 1.0000x reference)
"""Trainium2 Bass kernel for nn_EdgeClassifier (2x GraphSAGE mean-conv + edge MLP).

Design (v2): edges are laid out host-side into degree-sorted "passes":
each core's 6250 dst nodes are ranked by in-degree; pass p = ranks
[128p, 128p+128), one PSUM row per dst, C_p = max degree in the pass
(common across cores for SPMD). Segment-sum = C_p identity-stationary
bf16 matmuls accumulating in PSUM (~75ns/chunk); deg comes from a ones
column; deginv is a per-partition scalar; a transpose-matmul yields
aggT for the node-update GEMMs (all bf16). Phase C reuses the same
slot layout so Q[dst] is a per-pass 128x64 table (no 12.8MB gather);
per chunk one G^T=[P[src];ea] stationary matmul + per group a Q-inject
matmul assemble u in PSUM; ScalarE relu + DVE strided reduces produce
the +/- score sums.
"""

import numpy as np
import ml_dtypes
import concourse.mybir as mybir
import concourse.tile as tile
from concourse import bacc
from concourse.bass_utils import run_bass_kernel_spmd

F32 = mybir.dt.float32
BF16 = mybir.dt.bfloat16
AX = mybir.AluOpType
ACT = mybir.ActivationFunctionType

N_NODES = 50000
N_CORES = 8
OWN = N_NODES // N_CORES          # 6250
NPASS = (OWN + 127) // 128        # 49
NPAD = NPASS * 128                # 6272
HID = 64
EDIM = 16
GRP = 8                           # chunks per MLP group (phase C)

_CACHE = {}
LAST_HW_NS = 0
LAST_PHASE_NS = []
LAST_RUNS = []   # (builder, build_args, in_maps) for test-side HW timing


def bf16(x):
    return np.ascontiguousarray(np.asarray(x, np.float32).astype(ml_dtypes.bfloat16))


# ---------------------------------------------------------------- host plan

def make_plan(edge_index):
    src = np.asarray(edge_index[0], np.int64)
    dst = np.asarray(edge_index[1], np.int64)
    E = src.shape[0]
    deg = np.bincount(dst, minlength=N_NODES)
    core_of = dst // OWN
    # per-core rank of each own-node: sort by degree desc (stable)
    rank = np.empty(N_NODES, np.int64)      # rank within core, 0..OWN-1
    node_of_rank = np.empty((N_CORES, NPAD), np.int64)   # padded with -1
    node_of_rank.fill(-1)
    Cp_core = np.zeros((N_CORES, NPASS), np.int64)
    for c in range(N_CORES):
        lo, hi = c * OWN, (c + 1) * OWN
        order = np.argsort(-deg[lo:hi], kind="stable")
        rank[lo + order] = np.arange(OWN)
        node_of_rank[c, :OWN] = lo + order
        dsort = deg[lo + order]
        for p in range(NPASS):
            blk = dsort[p * 128:(p + 1) * 128]
            Cp_core[c, p] = blk.max() if len(blk) else 0
    Cp = np.maximum(Cp_core.max(axis=0), 1)
    cb = np.zeros(NPASS + 1, np.int64)
    cb[1:] = np.cumsum(Cp)
    NCH = int(cb[-1])
    # slot of each edge: order edges by dst, j = index within its dst group
    order_e = np.argsort(dst, kind="stable")
    ds = dst[order_e]
    # j = position within each dst run
    first = np.r_[True, ds[1:] != ds[:-1]]
    idx_of_first = np.flatnonzero(first)
    runlen_base = np.repeat(idx_of_first, np.diff(np.r_[idx_of_first, len(ds)]))
    j = np.arange(len(ds)) - runlen_base
    r_rank = rank[ds]
    p_of = r_rank // 128
    row = r_rank % 128
    col = cb[p_of] + j                      # chunk index within [0, NCH)
    core_e = core_of[order_e]
    return dict(E=E, deg=deg, rank=rank, node_of_rank=node_of_rank,
                Cp=Cp, cb=cb, NCH=NCH, order_e=order_e, src_e=src[order_e],
                core_e=core_e, row=row, col=col)


def build_msgs(plan, table_bf16, ones_col=True):
    """[core][128, NCH, 65] bf16 message tiles from a node table [N, 64]."""
    NCH = plan["NCH"]
    out = np.zeros((N_CORES, 128, NCH, 65), ml_dtypes.bfloat16)
    src_e, core_e = plan["src_e"], plan["core_e"]
    row, col = plan["row"], plan["col"]
    for c in range(N_CORES):
        m = core_e == c
        out[c, row[m], col[m], :64] = table_bf16[src_e[m]]
        if ones_col:
            out[c, row[m], col[m], 64] = 1.0
    return out


def build_rootT(plan, table_bf16):
    """[core][64, NPAD] bf16: node table transposed in rank order."""
    out = np.zeros((N_CORES, 64, NPAD), ml_dtypes.bfloat16)
    for c in range(N_CORES):
        nr = plan["node_of_rank"][c]
        v = nr >= 0
        out[c, :, v] = table_bf16[nr[v]]
    return out


def unsort_cols(plan, hT_sorted_list):
    """Inverse of rank ordering: [core][64, NPAD] -> full [N, 64] fp32."""
    full = np.zeros((N_NODES, 64), np.float32)
    for c in range(N_CORES):
        nr = plan["node_of_rank"][c]
        v = nr >= 0
        full[nr[v]] = np.asarray(hT_sorted_list[c], np.float32).T[v]
    return full


# ---------------------------------------------------------------- builders

def build_phase_ab(Cp, layer, repeat=1, stages=99, psum_bufs=2, dma_grp=64):
    NPASSL = len(Cp)
    NCH = int(np.sum(Cp))
    cb = np.zeros(NPASSL + 1, np.int64)
    cb[1:] = np.cumsum(Cp)

    nc = bacc.Bacc(None, target_bir_lowering=False)
    msgs = nc.dram_tensor("msgs", [128, NCH, 65], BF16, kind="ExternalInput")
    ident = nc.dram_tensor("ident", [128, 128], BF16, kind="ExternalInput")
    rootT = nc.dram_tensor("rootT", [64, NPAD], BF16, kind="ExternalInput")
    WS = nc.dram_tensor("WS", [128, 64], BF16, kind="ExternalInput")
    bl = nc.dram_tensor("bl", [64, 1], F32, kind="ExternalInput")
    hT_out = nc.dram_tensor("hT", [64, NPAD], BF16, kind="ExternalOutput")
    if layer == 2:
        PQW = nc.dram_tensor("PQW", [64, 128], BF16, kind="ExternalInput")
        bp2 = nc.dram_tensor("bp2", [128, 1], F32, kind="ExternalInput")
        PT_out = nc.dram_tensor("PT", [64, NPAD], BF16, kind="ExternalOutput")
        QT_out = nc.dram_tensor("QT", [64, NPAD], BF16, kind="ExternalOutput")

    dma_groups = []
    g0 = 0
    while g0 < NCH:
        dma_groups.append((g0, min(dma_grp, NCH - g0)))
        g0 += dma_grp

    with tile.TileContext(nc) as tc:
        with tc.tile_pool(name="const", bufs=1) as cp, \
             tc.tile_pool(name="big", bufs=1) as bigp, \
             tc.tile_pool(name="mg", bufs=3) as mgp, \
             tc.tile_pool(name="ss", bufs=3) as ssp, \
             tc.tile_pool(name="ps", bufs=psum_bufs, space="PSUM") as psp, \
             tc.tile_pool(name="ps2", bufs=2, space="PSUM") as ps2p, \
             tc.tile_pool(name="ps3", bufs=2, space="PSUM") as ps3p:

            id_t = cp.tile([128, 128], BF16)
            nc.sync.dma_start(id_t[:], ident[:])
            WS_t = cp.tile([128, 64], BF16)
            nc.sync.dma_start(WS_t[:], WS[:])
            bl_t = cp.tile([64, 1], F32)
            nc.sync.dma_start(bl_t[:], bl[:])
            XB = bigp.tile([128, NPAD], BF16)
            nc.sync.dma_start(XB[64:128, :], rootT[:])
            hT_sb = bigp.tile([64, NPAD], BF16)
            if stages < 99:
                nc.vector.memset(hT_sb[:], 0.0)
            if layer == 2:
                PQW_t = cp.tile([64, 128], BF16)
                nc.sync.dma_start(PQW_t[:], PQW[:])
                bp2_t = cp.tile([128, 1], F32)
                nc.sync.dma_start(bp2_t[:], bp2[:])
                PQ_sb = bigp.tile([128, NPAD], BF16)
                if stages < 99:
                    nc.vector.memset(PQ_sb[:], 0.0)

            SS = bigp.tile([128, NPASSL, 65], F32)
            sscall = bigp.tile([128, NPASSL, 64], BF16)

            def body():
                gi = 0
                mt = None
                mt_lo = mt_n = 0
                # sweep 1: scatter all passes, raw slot sums -> SS
                for p in range(NPASSL):
                    C = int(Cp[p])
                    pw = psp.tile([128, 65], F32, tag="pw")
                    for j in range(C):
                        ch = int(cb[p]) + j
                        if mt is None or ch >= mt_lo + mt_n:
                            lo, n = dma_groups[gi]
                            gi += 1
                            mt = mgp.tile([128, dma_grp, 65], BF16, tag="mt")
                            nc.sync.dma_start(mt[:, :n, :],
                                              msgs[:, lo:lo + n, :])
                            mt_lo, mt_n = lo, n
                        nc.tensor.matmul(pw[:], id_t[:],
                                         mt[:, ch - mt_lo, :],
                                         start=(j == 0), stop=(j == C - 1))
                    nc.vector.tensor_copy(SS[:, p, :], pw[:])
                if stages < 2:
                    nc.vector.tensor_copy(hT_sb[:, 0:NPASSL],
                                          SS[0:64, :, 0])
                    return
                # batched deginv scale (DVE)
                dvall = ssp.tile([128, NPASSL, 1], F32, tag="dvall")
                nc.vector.tensor_scalar(out=dvall[:], in0=SS[:, :, 64:65],
                                        scalar1=1.0, scalar2=None, op0=AX.max)
                nc.vector.reciprocal(dvall[:], dvall[:])
                nc.vector.tensor_tensor(
                    out=sscall[:], in0=SS[:, :, 0:64],
                    in1=dvall[:].broadcast_to([128, NPASSL, 64]),
                    op=AX.mult)
                # sweep 2: grouped transposes, then node updates
                TG = 4
                for p0 in range(0, NPASSL, TG):
                    tn = min(TG, NPASSL - p0)
                    pt = ps2p.tile([64, TG, 128], F32, tag="pt")
                    for t in range(tn):
                        nc.tensor.matmul(pt[:, t, :], sscall[:, p0 + t, :],
                                         id_t[:], start=True, stop=True)
                    nc.vector.tensor_copy(
                        XB[0:64, p0 * 128:(p0 + tn) * 128], pt[:, :tn, :])
                    if stages < 3:
                        continue
                    for t in range(tn):
                        p = p0 + t
                        # node update: [Wl;Wr]^T @ [aggT; rootT]
                        ph = ps3p.tile([64, 128], F32, tag="ph")
                        nc.tensor.matmul(ph[:], WS_t[:],
                                         XB[:, p * 128:(p + 1) * 128],
                                         start=True, stop=True)
                        nc.scalar.activation(hT_sb[:, p * 128:(p + 1) * 128],
                                             ph[:], ACT.Relu, bias=bl_t[:, 0:1])
                        if layer == 2:
                            pq = ps3p.tile([128, 128], F32, tag="pq")
                            nc.tensor.matmul(pq[:], PQW_t[:],
                                             hT_sb[:, p * 128:(p + 1) * 128],
                                             start=True, stop=True)
                            nc.vector.tensor_scalar(
                                out=PQ_sb[:, p * 128:(p + 1) * 128],
                                in0=pq[:], scalar1=bp2_t[:, 0:1],
                                scalar2=None, op0=AX.add)
                if stages < 3:
                    nc.vector.tensor_copy(hT_sb[:, 0:NPASSL],
                                          XB[0:64, 0:NPASSL])

            if repeat > 1:
                with tc.For_i(0, repeat):
                    body()
            else:
                body()

            nc.sync.dma_start(hT_out[:], hT_sb[:])
            if layer == 2:
                nc.sync.dma_start(PT_out[:], PQ_sb[0:64, :])
                nc.sync.dma_start(QT_out[:], PQ_sb[64:128, :])
    nc.compile()
    return nc


def build_phase_c(Cp, npos, bm2, repeat=1, stages=99, grp=GRP, psum_bufs=4):
    NPASSL = len(Cp)
    NCH = int(np.sum(Cp))
    cb = np.zeros(NPASSL + 1, np.int64)
    cb[1:] = np.cumsum(Cp)
    nneg = 64 - npos

    nc = bacc.Bacc(None, target_bir_lowering=False)
    G = nc.dram_tensor("G", [80, NCH, 128], BF16, kind="ExternalInput")
    Qp = nc.dram_tensor("Qp", [128, NPASSL, 64], BF16, kind="ExternalInput")
    M2 = nc.dram_tensor("M2", [80, 64], BF16, kind="ExternalInput")
    ident = nc.dram_tensor("ident", [128, 128], BF16, kind="ExternalInput")
    sc_out = nc.dram_tensor("scores", [128, NCH], F32, kind="ExternalOutput")

    dma_groups = []
    g0 = 0
    while g0 < NCH:
        dma_groups.append((g0, min(24, NCH - g0)))
        g0 += 24

    with tile.TileContext(nc) as tc:
        with tc.tile_pool(name="const", bufs=1) as cp, \
             tc.tile_pool(name="big", bufs=1) as bigp, \
             tc.tile_pool(name="mg", bufs=3) as mgp, \
             tc.tile_pool(name="qr", bufs=2) as qrp, \
             tc.tile_pool(name="ru", bufs=4) as rup, \
             tc.tile_pool(name="ps", bufs=psum_bufs, space="PSUM") as psp:

            id_t = cp.tile([128, 128], BF16)
            nc.sync.dma_start(id_t[:], ident[:])
            M2_t = cp.tile([80, 64], BF16)
            nc.sync.dma_start(M2_t[:], M2[:])
            Qp_t = bigp.tile([128, NPASSL, 64], BF16)
            nc.sync.dma_start(Qp_t[:], Qp[:])
            sc_sb = bigp.tile([128, NCH], F32)

            CMAX = int(max(Cp))
            qrep_all = bigp.tile([128, NPASSL, 8, 64], BF16)
            nc.vector.tensor_copy(
                qrep_all[:],
                Qp_t[:, :, None, :].broadcast_to([128, NPASSL, 8, 64]))

            def body():
                gi = 0
                gt = None
                gt_lo = gt_n = 0
                for p in range(NPASSL):
                    C = int(Cp[p])
                    ru = rup.tile([128, CMAX, 64], BF16, tag="ru")
                    for s0 in range(0, C, grp):
                        g = min(grp, C - s0)
                        pw = psp.tile([128, grp, 64], F32, tag="pw")
                        if stages >= 2:
                            for q0 in range(0, g, 8):
                                qn = min(8, g - q0)
                                nc.tensor.matmul(pw[:, q0:q0 + qn, :], id_t[:],
                                                 qrep_all[:, p, :qn, :],
                                                 start=True, stop=False,
                                                 skip_group_check=True)
                        for j in range(g):
                            ch = int(cb[p]) + s0 + j
                            if gt is None or ch >= gt_lo + gt_n:
                                lo, n = dma_groups[gi]
                                gi += 1
                                gt = mgp.tile([80, 24, 128], BF16, tag="gt")
                                nc.sync.dma_start(gt[:, :n, :],
                                                  G[:, lo:lo + n, :])
                                gt_lo, gt_n = lo, n
                            nc.tensor.matmul(pw[:, j, :],
                                             gt[:, ch - gt_lo, :], M2_t[:],
                                             start=(stages < 2), stop=True,
                                             skip_group_check=True)
                        c0 = int(cb[p]) + s0
                        if stages < 3:
                            nc.vector.tensor_copy(sc_sb[:, c0:c0 + g],
                                                  pw[:, :g, 0])
                            continue
                        nc.scalar.activation(ru[:, s0:s0 + g, :],
                                             pw[:, :g, :], ACT.Relu)
                    if stages < 3:
                        continue
                    c0 = int(cb[p])
                    if stages < 4:
                        nc.vector.tensor_copy(sc_sb[:, c0:c0 + C],
                                              ru[:, :C, 0])
                        continue
                    pos = rup.tile([128, CMAX], F32, tag="pos")
                    nc.vector.tensor_reduce(
                        pos[:, :C], ru[:, :C, 0:npos],
                        axis=mybir.AxisListType.X, op=AX.add)
                    neg = rup.tile([128, CMAX], F32, tag="neg")
                    nc.vector.tensor_reduce(
                        neg[:, :C], ru[:, :C, npos:64],
                        axis=mybir.AxisListType.X, op=AX.add)
                    nc.vector.tensor_tensor(
                        out=sc_sb[:, c0:c0 + C], in0=pos[:, :C],
                        in1=neg[:, :C], op=AX.subtract)
                nc.vector.tensor_scalar(out=sc_sb[:], in0=sc_sb[:],
                                        scalar1=float(bm2), scalar2=None,
                                        op0=AX.add)

            if repeat > 1:
                with tc.For_i(0, repeat):
                    body()
            else:
                body()
            nc.sync.dma_start(sc_out[:], sc_sb[:])
    nc.compile()
    return nc


# ---------------------------------------------------------------- pipeline

def _run(nc, in_maps):
    import time
    t0 = time.time()
    r = run_bass_kernel_spmd(nc, in_maps, core_ids=list(range(N_CORES)))
    LAST_PHASE_NS.append((time.time() - t0) * 1e9)
    return r.results


def kernel(x, edge_index, edge_attr, W1l, b1l, W1r, W2l, b2l, W2r,
           Wm1, bm1, Wm2, bm2):
    global LAST_HW_NS
    LAST_HW_NS = 0
    del LAST_PHASE_NS[:]
    del LAST_RUNS[:]
    x = np.asarray(x, np.float32)
    edge_attr = np.asarray(edge_attr, np.float32)
    Wm1 = np.asarray(Wm1, np.float32)
    Wm2 = np.asarray(Wm2, np.float32)
    plan = make_plan(edge_index)
    Cp = plan["Cp"]
    key = tuple(int(v) for v in Cp)
    ident = np.eye(128, dtype=np.float32).astype(ml_dtypes.bfloat16)

    # fold |Wm2| + sign permutation into edge-MLP weights
    w2 = Wm2[:, 0]
    D = np.abs(w2)
    perm = np.argsort(w2 <= 0, kind="stable")
    npos = int((w2 > 0).sum())
    A_ = bf16((Wm1[0:64] * D)[:, perm])
    B_ = bf16((Wm1[64:128] * D)[:, perm])
    C_ = bf16((Wm1[128:144] * D)[:, perm])
    bp_ = np.ascontiguousarray(
        (((np.asarray(bm1, np.float32) * D)[perm]) / 2.0)[:, None], np.float32)
    bm2f = float(np.asarray(bm2).reshape(-1)[0])

    # ---- phase A
    x_b = bf16(x)
    msgsA = build_msgs(plan, x_b)
    rootA = build_rootT(plan, x_b)
    if ("A", key) not in _CACHE:
        _CACHE[("A", key)] = build_phase_ab(Cp, layer=1)
    WS1 = bf16(np.concatenate([np.asarray(W1l, np.float32),
                               np.asarray(W1r, np.float32)], axis=0))
    mapsA = [dict(msgs=msgsA[c], ident=ident, rootT=rootA[c], WS=WS1,
                  bl=np.ascontiguousarray(np.asarray(b1l, np.float32)[:, None]))
             for c in range(N_CORES)]
    LAST_RUNS.append((build_phase_ab, dict(Cp=Cp, layer=1), mapsA))
    resA = _run(_CACHE[("A", key)], mapsA)
    h1 = unsort_cols(plan, [r["hT"] for r in resA])

    # ---- phase B
    h1_b = bf16(h1)
    msgsB = build_msgs(plan, h1_b)
    rootB = build_rootT(plan, h1_b)
    if ("B", key) not in _CACHE:
        _CACHE[("B", key)] = build_phase_ab(Cp, layer=2)
    WS2 = bf16(np.concatenate([np.asarray(W2l, np.float32),
                               np.asarray(W2r, np.float32)], axis=0))
    PQW = bf16(np.concatenate([A_.astype(np.float32),
                               B_.astype(np.float32)], axis=1))
    bp2 = np.ascontiguousarray(np.concatenate([bp_, bp_], axis=0), np.float32)
    mapsB = [dict(msgs=msgsB[c], ident=ident, rootT=rootB[c], WS=WS2,
                  bl=np.ascontiguousarray(np.asarray(b2l, np.float32)[:, None]),
                  PQW=PQW, bp2=bp2)
             for c in range(N_CORES)]
    LAST_RUNS.append((build_phase_ab, dict(Cp=Cp, layer=2), mapsB))
    resB = _run(_CACHE[("B", key)], mapsB)
    P = unsort_cols(plan, [r["PT"] for r in resB])
    Q = unsort_cols(plan, [r["QT"] for r in resB])

    # ---- phase C
    P_b = bf16(P)
    ea_b = bf16(edge_attr)
    NCH = plan["NCH"]
    Q_b = bf16(Q)
    M2 = np.zeros((80, 64), np.float32)
    M2[0:64] = np.eye(64)
    M2[64:80] = C_.astype(np.float32)
    M2 = bf16(M2)
    if ("C", key, npos, bm2f) not in _CACHE:
        _CACHE[("C", key, npos, bm2f)] = build_phase_c(Cp, npos, bm2f)
    src_e, core_e = plan["src_e"], plan["core_e"]
    row, col = plan["row"], plan["col"]
    oe = plan["order_e"]
    mapsC = []
    for c in range(N_CORES):
        m = core_e == c
        G = np.zeros((80, NCH, 128), ml_dtypes.bfloat16)
        G[0:64, col[m], row[m]] = P_b[src_e[m]].T
        G[64:80, col[m], row[m]] = ea_b[oe[m]].T
        Qp = np.zeros((128, NPASS, 64), ml_dtypes.bfloat16)
        nr = plan["node_of_rank"][c].reshape(NPASS, 128)
        for p in range(NPASS):
            v = nr[p] >= 0
            Qp[v, p, :] = Q_b[nr[p][v]]
        mapsC.append(dict(G=G, Qp=Qp, M2=M2, ident=ident))
    LAST_RUNS.append((build_phase_c, dict(Cp=Cp, npos=npos, bm2=bm2f), mapsC))
    resC = _run(_CACHE[("C", key, npos, bm2f)], mapsC)

    out = np.empty(plan["E"], np.float32)
    for c in range(N_CORES):
        m = core_e == c
        out[oe[m]] = resC[c]["scores"][row[m], col[m]]
    return out


# revision 6
# speedup vs baseline: 1.0084x; 1.0084x over previous
"""Trainium2 Bass kernel for nn_EdgeClassifier (2x GraphSAGE mean-conv + edge MLP).

Design (v2): edges are laid out host-side into degree-sorted "passes":
each core's 6250 dst nodes are ranked by in-degree; pass p = ranks
[128p, 128p+128), one PSUM row per dst, C_p = max degree in the pass
(common across cores for SPMD). Segment-sum = C_p identity-stationary
bf16 matmuls accumulating in PSUM (~75ns/chunk); deg comes from a ones
column; deginv is a per-partition scalar; a transpose-matmul yields
aggT for the node-update GEMMs (all bf16). Phase C reuses the same
slot layout so Q[dst] is a per-pass 128x64 table (no 12.8MB gather);
per chunk one G^T=[P[src];ea] stationary matmul + per group a Q-inject
matmul assemble u in PSUM; ScalarE relu + DVE strided reduces produce
the +/- score sums.
"""

import numpy as np
import ml_dtypes
import concourse.mybir as mybir
import concourse.tile as tile
from concourse import bacc
from concourse.bass_utils import run_bass_kernel_spmd

F32 = mybir.dt.float32
BF16 = mybir.dt.bfloat16
AX = mybir.AluOpType
ACT = mybir.ActivationFunctionType

N_NODES = 50000
N_CORES = 8
OWN = N_NODES // N_CORES          # 6250
NPASS = (OWN + 127) // 128        # 49
NPAD = NPASS * 128                # 6272
HID = 64
EDIM = 16
GRP = 8                           # chunks per MLP group (phase C)

_CACHE = {}
LAST_HW_NS = 0
LAST_PHASE_NS = []
LAST_RUNS = []   # (builder, build_args, in_maps) for test-side HW timing


def bf16(x):
    return np.ascontiguousarray(np.asarray(x, np.float32).astype(ml_dtypes.bfloat16))


# ---------------------------------------------------------------- host plan

def make_plan(edge_index):
    src = np.asarray(edge_index[0], np.int64)
    dst = np.asarray(edge_index[1], np.int64)
    E = src.shape[0]
    deg = np.bincount(dst, minlength=N_NODES)
    core_of = dst // OWN
    # per-core rank of each own-node: sort by degree desc (stable)
    rank = np.empty(N_NODES, np.int64)      # rank within core, 0..OWN-1
    node_of_rank = np.empty((N_CORES, NPAD), np.int64)   # padded with -1
    node_of_rank.fill(-1)
    Cp_core = np.zeros((N_CORES, NPASS), np.int64)
    for c in range(N_CORES):
        lo, hi = c * OWN, (c + 1) * OWN
        order = np.argsort(-deg[lo:hi], kind="stable")
        rank[lo + order] = np.arange(OWN)
        node_of_rank[c, :OWN] = lo + order
        dsort = deg[lo + order]
        for p in range(NPASS):
            blk = dsort[p * 128:(p + 1) * 128]
            Cp_core[c, p] = blk.max() if len(blk) else 0
    Cp = np.maximum(Cp_core.max(axis=0), 1)
    cb = np.zeros(NPASS + 1, np.int64)
    cb[1:] = np.cumsum(Cp)
    NCH = int(cb[-1])
    # slot of each edge: order edges by dst, j = index within its dst group
    order_e = np.argsort(dst, kind="stable")
    ds = dst[order_e]
    # j = position within each dst run
    first = np.r_[True, ds[1:] != ds[:-1]]
    idx_of_first = np.flatnonzero(first)
    runlen_base = np.repeat(idx_of_first, np.diff(np.r_[idx_of_first, len(ds)]))
    j = np.arange(len(ds)) - runlen_base
    r_rank = rank[ds]
    p_of = r_rank // 128
    row = r_rank % 128
    col = cb[p_of] + j                      # chunk index within [0, NCH)
    core_e = core_of[order_e]
    return dict(E=E, deg=deg, rank=rank, node_of_rank=node_of_rank,
                Cp=Cp, cb=cb, NCH=NCH, order_e=order_e, src_e=src[order_e],
                core_e=core_e, row=row, col=col)


def build_msgs(plan, table_bf16, ones_col=True):
    """[core][128, NCH, 65] bf16 message tiles from a node table [N, 64]."""
    NCH = plan["NCH"]
    out = np.zeros((N_CORES, 128, NCH, 65), ml_dtypes.bfloat16)
    src_e, core_e = plan["src_e"], plan["core_e"]
    row, col = plan["row"], plan["col"]
    for c in range(N_CORES):
        m = core_e == c
        out[c, row[m], col[m], :64] = table_bf16[src_e[m]]
        if ones_col:
            out[c, row[m], col[m], 64] = 1.0
    return out


def build_rootT(plan, table_bf16):
    """[core][64, NPAD] bf16: node table transposed in rank order."""
    out = np.zeros((N_CORES, 64, NPAD), ml_dtypes.bfloat16)
    for c in range(N_CORES):
        nr = plan["node_of_rank"][c]
        v = nr >= 0
        out[c, :, v] = table_bf16[nr[v]]
    return out


def unsort_cols(plan, hT_sorted_list):
    """Inverse of rank ordering: [core][64, NPAD] -> full [N, 64] fp32."""
    full = np.zeros((N_NODES, 64), np.float32)
    for c in range(N_CORES):
        nr = plan["node_of_rank"][c]
        v = nr >= 0
        full[nr[v]] = np.asarray(hT_sorted_list[c], np.float32).T[v]
    return full


# ---------------------------------------------------------------- builders

def build_phase_ab(Cp, layer, repeat=1, stages=99, psum_bufs=2, dma_grp=64):
    NPASSL = len(Cp)
    NCH = int(np.sum(Cp))
    cb = np.zeros(NPASSL + 1, np.int64)
    cb[1:] = np.cumsum(Cp)

    nc = bacc.Bacc(None, target_bir_lowering=False)
    msgs = nc.dram_tensor("msgs", [128, NCH, 65], BF16, kind="ExternalInput")
    ident = nc.dram_tensor("ident", [128, 128], BF16, kind="ExternalInput")
    rootT = nc.dram_tensor("rootT", [64, NPAD], BF16, kind="ExternalInput")
    WS = nc.dram_tensor("WS", [128, 64], BF16, kind="ExternalInput")
    bl = nc.dram_tensor("bl", [64, 1], F32, kind="ExternalInput")
    hT_out = nc.dram_tensor("hT", [64, NPAD], BF16, kind="ExternalOutput")
    if layer == 2:
        PQW = nc.dram_tensor("PQW", [64, 128], BF16, kind="ExternalInput")
        bp2 = nc.dram_tensor("bp2", [128, 1], F32, kind="ExternalInput")
        PT_out = nc.dram_tensor("PT", [64, NPAD], BF16, kind="ExternalOutput")
        QT_out = nc.dram_tensor("QT", [64, NPAD], BF16, kind="ExternalOutput")

    dma_groups = []
    g0 = 0
    while g0 < NCH:
        dma_groups.append((g0, min(dma_grp, NCH - g0)))
        g0 += dma_grp

    with tile.TileContext(nc) as tc:
        with tc.tile_pool(name="const", bufs=1) as cp, \
             tc.tile_pool(name="big", bufs=1) as bigp, \
             tc.tile_pool(name="mg", bufs=3) as mgp, \
             tc.tile_pool(name="ss", bufs=3) as ssp, \
             tc.tile_pool(name="ps", bufs=psum_bufs, space="PSUM") as psp, \
             tc.tile_pool(name="ps2", bufs=2, space="PSUM") as ps2p, \
             tc.tile_pool(name="ps3", bufs=2, space="PSUM") as ps3p:

            id_t = cp.tile([128, 128], BF16)
            nc.sync.dma_start(id_t[:], ident[:])
            WS_t = cp.tile([128, 64], BF16)
            nc.sync.dma_start(WS_t[:], WS[:])
            bl_t = cp.tile([64, 1], F32)
            nc.sync.dma_start(bl_t[:], bl[:])
            XB = bigp.tile([128, NPAD], BF16)
            nc.sync.dma_start(XB[64:128, :], rootT[:])
            hT_sb = bigp.tile([64, NPAD], BF16)
            if stages < 99:
                nc.vector.memset(hT_sb[:], 0.0)
            if layer == 2:
                PQW_t = cp.tile([64, 128], BF16)
                nc.sync.dma_start(PQW_t[:], PQW[:])
                bp2_t = cp.tile([128, 1], F32)
                nc.sync.dma_start(bp2_t[:], bp2[:])
                PQ_sb = bigp.tile([128, NPAD], BF16)
                if stages < 99:
                    nc.vector.memset(PQ_sb[:], 0.0)

            SS = bigp.tile([128, NPASSL, 65], F32)
            sscall = bigp.tile([128, NPASSL, 64], BF16)

            def body():
                gi = 0
                mt = None
                mt_lo = mt_n = 0
                # sweep 1: scatter all passes, raw slot sums -> SS
                for p in range(NPASSL):
                    C = int(Cp[p])
                    pw = psp.tile([128, 65], F32, tag="pw")
                    for j in range(C):
                        ch = int(cb[p]) + j
                        if mt is None or ch >= mt_lo + mt_n:
                            lo, n = dma_groups[gi]
                            gi += 1
                            mt = mgp.tile([128, dma_grp, 65], BF16, tag="mt")
                            nc.sync.dma_start(mt[:, :n, :],
                                              msgs[:, lo:lo + n, :])
                            mt_lo, mt_n = lo, n
                        nc.tensor.matmul(pw[:], id_t[:],
                                         mt[:, ch - mt_lo, :],
                                         start=(j == 0), stop=(j == C - 1))
                    nc.vector.tensor_copy(SS[:, p, :], pw[:])
                if stages < 2:
                    nc.vector.tensor_copy(hT_sb[:, 0:NPASSL],
                                          SS[0:64, :, 0])
                    return
                # batched deginv scale (DVE)
                dvall = ssp.tile([128, NPASSL, 1], F32, tag="dvall")
                nc.vector.tensor_scalar(out=dvall[:], in0=SS[:, :, 64:65],
                                        scalar1=1.0, scalar2=None, op0=AX.max)
                nc.vector.reciprocal(dvall[:], dvall[:])
                nc.vector.tensor_tensor(
                    out=sscall[:], in0=SS[:, :, 0:64],
                    in1=dvall[:].broadcast_to([128, NPASSL, 64]),
                    op=AX.mult)
                # sweep 2: grouped transposes, then node updates
                TG = 4
                for p0 in range(0, NPASSL, TG):
                    tn = min(TG, NPASSL - p0)
                    pt = ps2p.tile([64, TG, 128], F32, tag="pt")
                    for t in range(tn):
                        nc.tensor.matmul(pt[:, t, :], sscall[:, p0 + t, :],
                                         id_t[:], start=True, stop=True)
                    nc.vector.tensor_copy(
                        XB[0:64, p0 * 128:(p0 + tn) * 128], pt[:, :tn, :])
                    if stages < 3:
                        continue
                    for t in range(tn):
                        p = p0 + t
                        # node update: [Wl;Wr]^T @ [aggT; rootT]
                        ph = ps3p.tile([64, 128], F32, tag="ph")
                        nc.tensor.matmul(ph[:], WS_t[:],
                                         XB[:, p * 128:(p + 1) * 128],
                                         start=True, stop=True)
                        nc.scalar.activation(hT_sb[:, p * 128:(p + 1) * 128],
                                             ph[:], ACT.Relu, bias=bl_t[:, 0:1])
                if layer == 2 and stages >= 3:
                    # sweep 3: P/Q projections (hT_sb fully ready)
                    for p in range(NPASSL):
                        pq = ps3p.tile([128, 128], F32, tag="pq")
                        nc.tensor.matmul(pq[:], PQW_t[:],
                                         hT_sb[:, p * 128:(p + 1) * 128],
                                         start=True, stop=True)
                        nc.vector.tensor_scalar(
                            out=PQ_sb[:, p * 128:(p + 1) * 128],
                            in0=pq[:], scalar1=bp2_t[:, 0:1],
                            scalar2=None, op0=AX.add)
                if stages < 3:
                    nc.vector.tensor_copy(hT_sb[:, 0:NPASSL],
                                          XB[0:64, 0:NPASSL])

            if repeat > 1:
                with tc.For_i(0, repeat):
                    body()
            else:
                body()

            nc.sync.dma_start(hT_out[:], hT_sb[:])
            if layer == 2:
                nc.sync.dma_start(PT_out[:], PQ_sb[0:64, :])
                nc.sync.dma_start(QT_out[:], PQ_sb[64:128, :])
    nc.compile()
    return nc


def build_phase_c(Cp, npos, bm2, repeat=1, stages=99, grp=GRP, psum_bufs=4,
                  cg=48):
    NPASSL = len(Cp)
    NCH = int(np.sum(Cp))
    cb = np.zeros(NPASSL + 1, np.int64)
    cb[1:] = np.cumsum(Cp)
    nneg = 64 - npos

    nc = bacc.Bacc(None, target_bir_lowering=False)
    G = nc.dram_tensor("G", [80, NCH, 128], BF16, kind="ExternalInput")
    Qp = nc.dram_tensor("Qp", [128, NPASSL, 64], BF16, kind="ExternalInput")
    M2 = nc.dram_tensor("M2", [80, 64], BF16, kind="ExternalInput")
    ident = nc.dram_tensor("ident", [128, 128], BF16, kind="ExternalInput")
    sc_out = nc.dram_tensor("scores", [128, NCH], F32, kind="ExternalOutput")

    dma_groups = []
    g0 = 0
    while g0 < NCH:
        dma_groups.append((g0, min(cg, NCH - g0)))
        g0 += cg

    with tile.TileContext(nc) as tc:
        with tc.tile_pool(name="const", bufs=1) as cp, \
             tc.tile_pool(name="big", bufs=1) as bigp, \
             tc.tile_pool(name="mg", bufs=3) as mgp, \
             tc.tile_pool(name="qr", bufs=2) as qrp, \
             tc.tile_pool(name="ru", bufs=4) as rup, \
             tc.tile_pool(name="ps", bufs=psum_bufs, space="PSUM") as psp:

            id_t = cp.tile([128, 128], BF16)
            nc.sync.dma_start(id_t[:], ident[:])
            M2_t = cp.tile([80, 64], BF16)
            nc.sync.dma_start(M2_t[:], M2[:])
            Qp_t = bigp.tile([128, NPASSL, 64], BF16)
            nc.sync.dma_start(Qp_t[:], Qp[:])
            sc_sb = bigp.tile([128, NCH], F32)

            CMAX = int(max(Cp))
            qrep_all = bigp.tile([128, NPASSL, 8, 64], BF16)
            nc.vector.tensor_copy(
                qrep_all[:],
                Qp_t[:, :, None, :].broadcast_to([128, NPASSL, 8, 64]))

            def body():
                gi = 0
                gt = None
                gt_lo = gt_n = 0
                for p in range(NPASSL):
                    C = int(Cp[p])
                    ru = rup.tile([128, CMAX, 64], BF16, tag="ru")
                    for s0 in range(0, C, grp):
                        g = min(grp, C - s0)
                        pw = psp.tile([128, grp, 64], F32, tag="pw")
                        if stages >= 2:
                            for q0 in range(0, g, 8):
                                qn = min(8, g - q0)
                                nc.tensor.matmul(pw[:, q0:q0 + qn, :], id_t[:],
                                                 qrep_all[:, p, :qn, :],
                                                 start=True, stop=False,
                                                 skip_group_check=True)
                        for j in range(g):
                            ch = int(cb[p]) + s0 + j
                            if gt is None or ch >= gt_lo + gt_n:
                                lo, n = dma_groups[gi]
                                gi += 1
                                gt = mgp.tile([80, cg, 128], BF16, tag="gt")
                                nc.sync.dma_start(gt[:, :n, :],
                                                  G[:, lo:lo + n, :])
                                gt_lo, gt_n = lo, n
                            nc.tensor.matmul(pw[:, j, :],
                                             gt[:, ch - gt_lo, :], M2_t[:],
                                             start=(stages < 2), stop=True,
                                             skip_group_check=True)
                        c0 = int(cb[p]) + s0
                        if stages < 3:
                            nc.vector.tensor_copy(sc_sb[:, c0:c0 + g],
                                                  pw[:, :g, 0])
                            continue
                        nc.scalar.activation(ru[:, s0:s0 + g, :],
                                             pw[:, :g, :], ACT.Relu)
                    if stages < 3:
                        continue
                    c0 = int(cb[p])
                    if stages < 4:
                        nc.vector.tensor_copy(sc_sb[:, c0:c0 + C],
                                              ru[:, :C, 0])
                        continue
                    pos = rup.tile([128, CMAX], F32, tag="pos")
                    nc.vector.tensor_reduce(
                        pos[:, :C], ru[:, :C, 0:npos],
                        axis=mybir.AxisListType.X, op=AX.add)
                    neg = rup.tile([128, CMAX], F32, tag="neg")
                    nc.vector.tensor_reduce(
                        neg[:, :C], ru[:, :C, npos:64],
                        axis=mybir.AxisListType.X, op=AX.add)
                    nc.vector.tensor_tensor(
                        out=sc_sb[:, c0:c0 + C], in0=pos[:, :C],
                        in1=neg[:, :C], op=AX.subtract)
                nc.vector.tensor_scalar(out=sc_sb[:], in0=sc_sb[:],
                                        scalar1=float(bm2), scalar2=None,
                                        op0=AX.add)

            if repeat > 1:
                with tc.For_i(0, repeat):
                    body()
            else:
                body()
            nc.sync.dma_start(sc_out[:], sc_sb[:])
    nc.compile()
    return nc


# ---------------------------------------------------------------- pipeline

def _run(nc, in_maps):
    import time
    t0 = time.time()
    r = run_bass_kernel_spmd(nc, in_maps, core_ids=list(range(N_CORES)))
    LAST_PHASE_NS.append((time.time() - t0) * 1e9)
    return r.results


def kernel(x, edge_index, edge_attr, W1l, b1l, W1r, W2l, b2l, W2r,
           Wm1, bm1, Wm2, bm2):
    global LAST_HW_NS
    LAST_HW_NS = 0
    del LAST_PHASE_NS[:]
    del LAST_RUNS[:]
    x = np.asarray(x, np.float32)
    edge_attr = np.asarray(edge_attr, np.float32)
    Wm1 = np.asarray(Wm1, np.float32)
    Wm2 = np.asarray(Wm2, np.float32)
    plan = make_plan(edge_index)
    Cp = plan["Cp"]
    key = tuple(int(v) for v in Cp)
    ident = np.eye(128, dtype=np.float32).astype(ml_dtypes.bfloat16)

    # fold |Wm2| + sign permutation into edge-MLP weights
    w2 = Wm2[:, 0]
    D = np.abs(w2)
    perm = np.argsort(w2 <= 0, kind="stable")
    npos = int((w2 > 0).sum())
    A_ = bf16((Wm1[0:64] * D)[:, perm])
    B_ = bf16((Wm1[64:128] * D)[:, perm])
    C_ = bf16((Wm1[128:144] * D)[:, perm])
    bp_ = np.ascontiguousarray(
        (((np.asarray(bm1, np.float32) * D)[perm]) / 2.0)[:, None], np.float32)
    bm2f = float(np.asarray(bm2).reshape(-1)[0])

    # ---- phase A
    x_b = bf16(x)
    msgsA = build_msgs(plan, x_b)
    rootA = build_rootT(plan, x_b)
    if ("A", key) not in _CACHE:
        _CACHE[("A", key)] = build_phase_ab(Cp, layer=1)
    WS1 = bf16(np.concatenate([np.asarray(W1l, np.float32),
                               np.asarray(W1r, np.float32)], axis=0))
    mapsA = [dict(msgs=msgsA[c], ident=ident, rootT=rootA[c], WS=WS1,
                  bl=np.ascontiguousarray(np.asarray(b1l, np.float32)[:, None]))
             for c in range(N_CORES)]
    LAST_RUNS.append((build_phase_ab, dict(Cp=Cp, layer=1), mapsA))
    resA = _run(_CACHE[("A", key)], mapsA)
    h1 = unsort_cols(plan, [r["hT"] for r in resA])

    # ---- phase B
    h1_b = bf16(h1)
    msgsB = build_msgs(plan, h1_b)
    rootB = build_rootT(plan, h1_b)
    if ("B", key) not in _CACHE:
        _CACHE[("B", key)] = build_phase_ab(Cp, layer=2)
    WS2 = bf16(np.concatenate([np.asarray(W2l, np.float32),
                               np.asarray(W2r, np.float32)], axis=0))
    PQW = bf16(np.concatenate([A_.astype(np.float32),
                               B_.astype(np.float32)], axis=1))
    bp2 = np.ascontiguousarray(np.concatenate([bp_, bp_], axis=0), np.float32)
    mapsB = [dict(msgs=msgsB[c], ident=ident, rootT=rootB[c], WS=WS2,
                  bl=np.ascontiguousarray(np.asarray(b2l, np.float32)[:, None]),
                  PQW=PQW, bp2=bp2)
             for c in range(N_CORES)]
    LAST_RUNS.append((build_phase_ab, dict(Cp=Cp, layer=2), mapsB))
    resB = _run(_CACHE[("B", key)], mapsB)
    P = unsort_cols(plan, [r["PT"] for r in resB])
    Q = unsort_cols(plan, [r["QT"] for r in resB])

    # ---- phase C
    P_b = bf16(P)
    ea_b = bf16(edge_attr)
    NCH = plan["NCH"]
    Q_b = bf16(Q)
    M2 = np.zeros((80, 64), np.float32)
    M2[0:64] = np.eye(64)
    M2[64:80] = C_.astype(np.float32)
    M2 = bf16(M2)
    if ("C", key, npos, bm2f) not in _CACHE:
        _CACHE[("C", key, npos, bm2f)] = build_phase_c(Cp, npos, bm2f)
    src_e, core_e = plan["src_e"], plan["core_e"]
    row, col = plan["row"], plan["col"]
    oe = plan["order_e"]
    mapsC = []
    for c in range(N_CORES):
        m = core_e == c
        G = np.zeros((80, NCH, 128), ml_dtypes.bfloat16)
        G[0:64, col[m], row[m]] = P_b[src_e[m]].T
        G[64:80, col[m], row[m]] = ea_b[oe[m]].T
        Qp = np.zeros((128, NPASS, 64), ml_dtypes.bfloat16)
        nr = plan["node_of_rank"][c].reshape(NPASS, 128)
        for p in range(NPASS):
            v = nr[p] >= 0
            Qp[v, p, :] = Q_b[nr[p][v]]
        mapsC.append(dict(G=G, Qp=Qp, M2=M2, ident=ident))
    LAST_RUNS.append((build_phase_c, dict(Cp=Cp, npos=npos, bm2=bm2f), mapsC))
    resC = _run(_CACHE[("C", key, npos, bm2f)], mapsC)

    out = np.empty(plan["E"], np.float32)
    for c in range(N_CORES):
        m = core_e == c
        out[oe[m]] = resC[c]["scores"][row[m], col[m]]
    return out


# revision 7
# speedup vs baseline: 1.1929x; 1.1829x over previous
"""Trainium2 Bass kernel for nn_EdgeClassifier (2x GraphSAGE mean-conv + edge MLP).

Design (v2): edges are laid out host-side into degree-sorted "passes":
each core's 6250 dst nodes are ranked by in-degree; pass p = ranks
[128p, 128p+128), one PSUM row per dst, C_p = max degree in the pass
(common across cores for SPMD). Segment-sum = C_p identity-stationary
bf16 matmuls accumulating in PSUM (~75ns/chunk); deg comes from a ones
column; deginv is a per-partition scalar; a transpose-matmul yields
aggT for the node-update GEMMs (all bf16). Phase C reuses the same
slot layout so Q[dst] is a per-pass 128x64 table (no 12.8MB gather);
per chunk one G^T=[P[src];ea] stationary matmul + per group a Q-inject
matmul assemble u in PSUM; ScalarE relu + DVE strided reduces produce
the +/- score sums.
"""

import numpy as np
import ml_dtypes
import concourse.mybir as mybir
import concourse.tile as tile
from concourse import bacc
from concourse.bass_utils import run_bass_kernel_spmd

F32 = mybir.dt.float32
BF16 = mybir.dt.bfloat16
AX = mybir.AluOpType
ACT = mybir.ActivationFunctionType

N_NODES = 50000
N_CORES = 8
OWN = N_NODES // N_CORES          # 6250
NPASS = (OWN + 127) // 128        # 49
NPAD = NPASS * 128                # 6272
HID = 64
EDIM = 16
GRP = 8                           # chunks per MLP group (phase C)

_CACHE = {}
LAST_HW_NS = 0
LAST_PHASE_NS = []
LAST_RUNS = []   # (builder, build_args, in_maps) for test-side HW timing


def bf16(x):
    return np.ascontiguousarray(np.asarray(x, np.float32).astype(ml_dtypes.bfloat16))


# ---------------------------------------------------------------- host plan

def make_plan(edge_index):
    src = np.asarray(edge_index[0], np.int64)
    dst = np.asarray(edge_index[1], np.int64)
    E = src.shape[0]
    deg = np.bincount(dst, minlength=N_NODES)
    core_of = dst // OWN
    # per-core rank of each own-node: sort by degree desc (stable)
    rank = np.empty(N_NODES, np.int64)      # rank within core, 0..OWN-1
    node_of_rank = np.empty((N_CORES, NPAD), np.int64)   # padded with -1
    node_of_rank.fill(-1)
    Cp_core = np.zeros((N_CORES, NPASS), np.int64)
    for c in range(N_CORES):
        lo, hi = c * OWN, (c + 1) * OWN
        order = np.argsort(-deg[lo:hi], kind="stable")
        rank[lo + order] = np.arange(OWN)
        node_of_rank[c, :OWN] = lo + order
        dsort = deg[lo + order]
        for p in range(NPASS):
            blk = dsort[p * 128:(p + 1) * 128]
            Cp_core[c, p] = blk.max() if len(blk) else 0
    Cp = np.maximum(Cp_core.max(axis=0), 1)
    cb = np.zeros(NPASS + 1, np.int64)
    cb[1:] = np.cumsum(Cp)
    NCH = int(cb[-1])
    # slot of each edge: order edges by dst, j = index within its dst group
    order_e = np.argsort(dst, kind="stable")
    ds = dst[order_e]
    # j = position within each dst run
    first = np.r_[True, ds[1:] != ds[:-1]]
    idx_of_first = np.flatnonzero(first)
    runlen_base = np.repeat(idx_of_first, np.diff(np.r_[idx_of_first, len(ds)]))
    j = np.arange(len(ds)) - runlen_base
    r_rank = rank[ds]
    p_of = r_rank // 128
    row = r_rank % 128
    col = cb[p_of] + j                      # chunk index within [0, NCH)
    core_e = core_of[order_e]
    return dict(E=E, deg=deg, rank=rank, node_of_rank=node_of_rank,
                Cp=Cp, cb=cb, NCH=NCH, order_e=order_e, src_e=src[order_e],
                core_e=core_e, row=row, col=col)


def build_msgs(plan, table_bf16, ones_col=True):
    """[core][128, NCH, 65] bf16 message tiles from a node table [N, 64]."""
    NCH = plan["NCH"]
    out = np.zeros((N_CORES, 128, NCH, 65), ml_dtypes.bfloat16)
    src_e, core_e = plan["src_e"], plan["core_e"]
    row, col = plan["row"], plan["col"]
    for c in range(N_CORES):
        m = core_e == c
        out[c, row[m], col[m], :64] = table_bf16[src_e[m]]
        if ones_col:
            out[c, row[m], col[m], 64] = 1.0
    return out


def build_rootT(plan, table_bf16):
    """[core][64, NPAD] bf16: node table transposed in rank order."""
    out = np.zeros((N_CORES, 64, NPAD), ml_dtypes.bfloat16)
    for c in range(N_CORES):
        nr = plan["node_of_rank"][c]
        v = nr >= 0
        out[c, :, v] = table_bf16[nr[v]]
    return out


def unsort_cols(plan, hT_sorted_list):
    """Inverse of rank ordering: [core][64, NPAD] -> full [N, 64] fp32."""
    full = np.zeros((N_NODES, 64), np.float32)
    for c in range(N_CORES):
        nr = plan["node_of_rank"][c]
        v = nr >= 0
        full[nr[v]] = np.asarray(hT_sorted_list[c], np.float32).T[v]
    return full


# ---------------------------------------------------------------- builders

def build_phase_ab(Cp, layer, repeat=1, stages=99, psum_bufs=2, dma_grp=64):
    NPASSL = len(Cp)
    NCH = int(np.sum(Cp))
    cb = np.zeros(NPASSL + 1, np.int64)
    cb[1:] = np.cumsum(Cp)

    nc = bacc.Bacc(None, target_bir_lowering=False)
    msgs = nc.dram_tensor("msgs", [128, NCH, 65], BF16, kind="ExternalInput")
    ident = nc.dram_tensor("ident", [128, 128], BF16, kind="ExternalInput")
    rootT = nc.dram_tensor("rootT", [64, NPAD], BF16, kind="ExternalInput")
    WS = nc.dram_tensor("WS", [128, 64], BF16, kind="ExternalInput")
    bl = nc.dram_tensor("bl", [64, 1], F32, kind="ExternalInput")
    hT_out = nc.dram_tensor("hT", [64, NPAD], BF16, kind="ExternalOutput")
    if layer == 2:
        PQW = nc.dram_tensor("PQW", [64, 128], BF16, kind="ExternalInput")
        bp2 = nc.dram_tensor("bp2", [128, 1], F32, kind="ExternalInput")
        PT_out = nc.dram_tensor("PT", [64, NPAD], BF16, kind="ExternalOutput")
        QT_out = nc.dram_tensor("QT", [64, NPAD], BF16, kind="ExternalOutput")

    dma_groups = []
    g0 = 0
    while g0 < NCH:
        dma_groups.append((g0, min(dma_grp, NCH - g0)))
        g0 += dma_grp

    with tile.TileContext(nc) as tc:
        with tc.tile_pool(name="const", bufs=1) as cp, \
             tc.tile_pool(name="big", bufs=1) as bigp, \
             tc.tile_pool(name="mg", bufs=3) as mgp, \
             tc.tile_pool(name="ss", bufs=3) as ssp, \
             tc.tile_pool(name="ps", bufs=psum_bufs, space="PSUM") as psp, \
             tc.tile_pool(name="ps2", bufs=2, space="PSUM") as ps2p, \
             tc.tile_pool(name="ps3", bufs=2, space="PSUM") as ps3p:

            id_t = cp.tile([128, 128], BF16)
            nc.sync.dma_start(id_t[:], ident[:])
            WS_t = cp.tile([128, 64], BF16)
            nc.sync.dma_start(WS_t[:], WS[:])
            bl_t = cp.tile([64, 1], F32)
            nc.sync.dma_start(bl_t[:], bl[:])
            XB = bigp.tile([128, NPAD], BF16)
            nc.sync.dma_start(XB[64:128, :], rootT[:])
            hT_sb = bigp.tile([64, NPAD], BF16)
            if stages < 99:
                nc.vector.memset(hT_sb[:], 0.0)
            if layer == 2:
                PQW_t = cp.tile([64, 128], BF16)
                nc.sync.dma_start(PQW_t[:], PQW[:])
                bp2_t = cp.tile([128, 1], F32)
                nc.sync.dma_start(bp2_t[:], bp2[:])
                PQ_sb = bigp.tile([128, NPAD], BF16)
                if stages < 99:
                    nc.vector.memset(PQ_sb[:], 0.0)

            SS = bigp.tile([128, NPASSL, 65], F32)
            sscall = bigp.tile([128, NPASSL, 64], BF16)

            def body():
                gi = 0
                mt = None
                mt_lo = mt_n = 0
                PG = 8
                for p0 in range(0, NPASSL, PG):
                    pn = min(PG, NPASSL - p0)
                    # scatter these passes -> SS
                    for t in range(pn):
                        p = p0 + t
                        C = int(Cp[p])
                        pw = psp.tile([128, 65], F32, tag="pw")
                        for j in range(C):
                            ch = int(cb[p]) + j
                            if mt is None or ch >= mt_lo + mt_n:
                                lo, n = dma_groups[gi]
                                gi += 1
                                mt = mgp.tile([128, dma_grp, 65], BF16,
                                              tag="mt")
                                nc.sync.dma_start(mt[:, :n, :],
                                                  msgs[:, lo:lo + n, :])
                                mt_lo, mt_n = lo, n
                            nc.tensor.matmul(pw[:], id_t[:],
                                             mt[:, ch - mt_lo, :],
                                             start=(j == 0),
                                             stop=(j == C - 1))
                        nc.vector.tensor_copy(SS[:, p, :], pw[:])
                    if stages < 2:
                        continue
                    # batched deginv scale for the group (DVE)
                    dv = ssp.tile([128, PG, 1], F32, tag="dv")
                    nc.vector.tensor_scalar(out=dv[:, :pn, :],
                                            in0=SS[:, p0:p0 + pn, 64:65],
                                            scalar1=1.0, scalar2=None,
                                            op0=AX.max)
                    nc.vector.reciprocal(dv[:, :pn, :], dv[:, :pn, :])
                    nc.vector.tensor_tensor(
                        out=sscall[:, p0:p0 + pn, :],
                        in0=SS[:, p0:p0 + pn, 0:64],
                        in1=dv[:, :pn, :].broadcast_to([128, pn, 64]),
                        op=AX.mult)
                    # transposes in sub-groups of 4 (PSUM bank limit)
                    for q0 in range(0, pn, 4):
                        qn = min(4, pn - q0)
                        pt = ps2p.tile([64, 4, 128], F32, tag="pt")
                        for t in range(qn):
                            nc.tensor.matmul(pt[:, t, :],
                                             sscall[:, p0 + q0 + t, :],
                                             id_t[:], start=True, stop=True)
                        nc.vector.tensor_copy(
                            XB[0:64, (p0 + q0) * 128:(p0 + q0 + qn) * 128],
                            pt[:, :qn, :])
                    if stages < 3:
                        continue
                    for t in range(pn):
                        p = p0 + t
                        ph = ps3p.tile([64, 128], F32, tag="ph")
                        nc.tensor.matmul(ph[:], WS_t[:],
                                         XB[:, p * 128:(p + 1) * 128],
                                         start=True, stop=True)
                        nc.scalar.activation(hT_sb[:, p * 128:(p + 1) * 128],
                                             ph[:], ACT.Relu,
                                             bias=bl_t[:, 0:1])
                if stages < 2:
                    nc.vector.tensor_copy(hT_sb[:, 0:NPASSL],
                                          SS[0:64, :, 0])
                    return
                if stages < 3:
                    nc.vector.tensor_copy(hT_sb[:, 0:NPASSL],
                                          XB[0:64, 0:NPASSL])
                    return
                if layer == 2:
                    # final sweep: P/Q projections (hT_sb fully ready)
                    for p in range(NPASSL):
                        pq = ps3p.tile([128, 128], F32, tag="pq")
                        nc.tensor.matmul(pq[:], PQW_t[:],
                                         hT_sb[:, p * 128:(p + 1) * 128],
                                         start=True, stop=True)
                        nc.vector.tensor_scalar(
                            out=PQ_sb[:, p * 128:(p + 1) * 128],
                            in0=pq[:], scalar1=bp2_t[:, 0:1],
                            scalar2=None, op0=AX.add)

            if repeat > 1:
                with tc.For_i(0, repeat):
                    body()
            else:
                body()

            nc.sync.dma_start(hT_out[:], hT_sb[:])
            if layer == 2:
                nc.sync.dma_start(PT_out[:], PQ_sb[0:64, :])
                nc.sync.dma_start(QT_out[:], PQ_sb[64:128, :])
    nc.compile()
    return nc


def build_phase_c(Cp, npos, bm2, repeat=1, stages=99, grp=GRP, psum_bufs=4,
                  cg=48):
    NPASSL = len(Cp)
    NCH = int(np.sum(Cp))
    cb = np.zeros(NPASSL + 1, np.int64)
    cb[1:] = np.cumsum(Cp)
    nneg = 64 - npos

    nc = bacc.Bacc(None, target_bir_lowering=False)
    G = nc.dram_tensor("G", [80, NCH, 128], BF16, kind="ExternalInput")
    Qp = nc.dram_tensor("Qp", [128, NPASSL, 64], BF16, kind="ExternalInput")
    M2 = nc.dram_tensor("M2", [80, 64], BF16, kind="ExternalInput")
    ident = nc.dram_tensor("ident", [128, 128], BF16, kind="ExternalInput")
    sc_out = nc.dram_tensor("scores", [128, NCH], F32, kind="ExternalOutput")

    dma_groups = []
    g0 = 0
    while g0 < NCH:
        dma_groups.append((g0, min(cg, NCH - g0)))
        g0 += cg

    with tile.TileContext(nc) as tc:
        with tc.tile_pool(name="const", bufs=1) as cp, \
             tc.tile_pool(name="big", bufs=1) as bigp, \
             tc.tile_pool(name="mg", bufs=3) as mgp, \
             tc.tile_pool(name="qr", bufs=2) as qrp, \
             tc.tile_pool(name="ru", bufs=4) as rup, \
             tc.tile_pool(name="ps", bufs=psum_bufs, space="PSUM") as psp:

            id_t = cp.tile([128, 128], BF16)
            nc.sync.dma_start(id_t[:], ident[:])
            M2_t = cp.tile([80, 64], BF16)
            nc.sync.dma_start(M2_t[:], M2[:])
            Qp_t = bigp.tile([128, NPASSL, 64], BF16)
            nc.sync.dma_start(Qp_t[:], Qp[:])
            sc_sb = bigp.tile([128, NCH], F32)

            CMAX = int(max(Cp))
            qrep_all = bigp.tile([128, NPASSL, 8, 64], BF16)
            nc.vector.tensor_copy(
                qrep_all[:],
                Qp_t[:, :, None, :].broadcast_to([128, NPASSL, 8, 64]))

            def body():
                gi = 0
                gt = None
                gt_lo = gt_n = 0
                for p in range(NPASSL):
                    C = int(Cp[p])
                    ru = rup.tile([128, CMAX, 64], BF16, tag="ru")
                    for s0 in range(0, C, grp):
                        g = min(grp, C - s0)
                        pw = psp.tile([128, grp, 64], F32, tag="pw")
                        if stages >= 2:
                            for q0 in range(0, g, 8):
                                qn = min(8, g - q0)
                                nc.tensor.matmul(pw[:, q0:q0 + qn, :], id_t[:],
                                                 qrep_all[:, p, :qn, :],
                                                 start=True, stop=False,
                                                 skip_group_check=True)
                        for j in range(g):
                            ch = int(cb[p]) + s0 + j
                            if gt is None or ch >= gt_lo + gt_n:
                                lo, n = dma_groups[gi]
                                gi += 1
                                gt = mgp.tile([80, cg, 128], BF16, tag="gt")
                                nc.sync.dma_start(gt[:, :n, :],
                                                  G[:, lo:lo + n, :])
                                gt_lo, gt_n = lo, n
                            nc.tensor.matmul(pw[:, j, :],
                                             gt[:, ch - gt_lo, :], M2_t[:],
                                             start=(stages < 2), stop=True,
                                             skip_group_check=True)
                        c0 = int(cb[p]) + s0
                        if stages < 3:
                            nc.vector.tensor_copy(sc_sb[:, c0:c0 + g],
                                                  pw[:, :g, 0])
                            continue
                        nc.scalar.activation(ru[:, s0:s0 + g, :],
                                             pw[:, :g, :], ACT.Relu)
                    if stages < 3:
                        continue
                    c0 = int(cb[p])
                    if stages < 4:
                        nc.vector.tensor_copy(sc_sb[:, c0:c0 + C],
                                              ru[:, :C, 0])
                        continue
                    pos = rup.tile([128, CMAX], F32, tag="pos")
                    nc.vector.tensor_reduce(
                        pos[:, :C], ru[:, :C, 0:npos],
                        axis=mybir.AxisListType.X, op=AX.add)
                    neg = rup.tile([128, CMAX], F32, tag="neg")
                    nc.vector.tensor_reduce(
                        neg[:, :C], ru[:, :C, npos:64],
                        axis=mybir.AxisListType.X, op=AX.add)
                    nc.vector.tensor_tensor(
                        out=sc_sb[:, c0:c0 + C], in0=pos[:, :C],
                        in1=neg[:, :C], op=AX.subtract)
                nc.vector.tensor_scalar(out=sc_sb[:], in0=sc_sb[:],
                                        scalar1=float(bm2), scalar2=None,
                                        op0=AX.add)

            if repeat > 1:
                with tc.For_i(0, repeat):
                    body()
            else:
                body()
            nc.sync.dma_start(sc_out[:], sc_sb[:])
    nc.compile()
    return nc


# ---------------------------------------------------------------- pipeline

def _run(nc, in_maps):
    import time
    t0 = time.time()
    r = run_bass_kernel_spmd(nc, in_maps, core_ids=list(range(N_CORES)))
    LAST_PHASE_NS.append((time.time() - t0) * 1e9)
    return r.results


def kernel(x, edge_index, edge_attr, W1l, b1l, W1r, W2l, b2l, W2r,
           Wm1, bm1, Wm2, bm2):
    global LAST_HW_NS
    LAST_HW_NS = 0
    del LAST_PHASE_NS[:]
    del LAST_RUNS[:]
    x = np.asarray(x, np.float32)
    edge_attr = np.asarray(edge_attr, np.float32)
    Wm1 = np.asarray(Wm1, np.float32)
    Wm2 = np.asarray(Wm2, np.float32)
    plan = make_plan(edge_index)
    Cp = plan["Cp"]
    key = tuple(int(v) for v in Cp)
    ident = np.eye(128, dtype=np.float32).astype(ml_dtypes.bfloat16)

    # fold |Wm2| + sign permutation into edge-MLP weights
    w2 = Wm2[:, 0]
    D = np.abs(w2)
    perm = np.argsort(w2 <= 0, kind="stable")
    npos = int((w2 > 0).sum())
    A_ = bf16((Wm1[0:64] * D)[:, perm])
    B_ = bf16((Wm1[64:128] * D)[:, perm])
    C_ = bf16((Wm1[128:144] * D)[:, perm])
    bp_ = np.ascontiguousarray(
        (((np.asarray(bm1, np.float32) * D)[perm]) / 2.0)[:, None], np.float32)
    bm2f = float(np.asarray(bm2).reshape(-1)[0])

    # ---- phase A
    x_b = bf16(x)
    msgsA = build_msgs(plan, x_b)
    rootA = build_rootT(plan, x_b)
    if ("A", key) not in _CACHE:
        _CACHE[("A", key)] = build_phase_ab(Cp, layer=1)
    WS1 = bf16(np.concatenate([np.asarray(W1l, np.float32),
                               np.asarray(W1r, np.float32)], axis=0))
    mapsA = [dict(msgs=msgsA[c], ident=ident, rootT=rootA[c], WS=WS1,
                  bl=np.ascontiguousarray(np.asarray(b1l, np.float32)[:, None]))
             for c in range(N_CORES)]
    LAST_RUNS.append((build_phase_ab, dict(Cp=Cp, layer=1), mapsA))
    resA = _run(_CACHE[("A", key)], mapsA)
    h1 = unsort_cols(plan, [r["hT"] for r in resA])

    # ---- phase B
    h1_b = bf16(h1)
    msgsB = build_msgs(plan, h1_b)
    rootB = build_rootT(plan, h1_b)
    if ("B", key) not in _CACHE:
        _CACHE[("B", key)] = build_phase_ab(Cp, layer=2)
    WS2 = bf16(np.concatenate([np.asarray(W2l, np.float32),
                               np.asarray(W2r, np.float32)], axis=0))
    PQW = bf16(np.concatenate([A_.astype(np.float32),
                               B_.astype(np.float32)], axis=1))
    bp2 = np.ascontiguousarray(np.concatenate([bp_, bp_], axis=0), np.float32)
    mapsB = [dict(msgs=msgsB[c], ident=ident, rootT=rootB[c], WS=WS2,
                  bl=np.ascontiguousarray(np.asarray(b2l, np.float32)[:, None]),
                  PQW=PQW, bp2=bp2)
             for c in range(N_CORES)]
    LAST_RUNS.append((build_phase_ab, dict(Cp=Cp, layer=2), mapsB))
    resB = _run(_CACHE[("B", key)], mapsB)
    P = unsort_cols(plan, [r["PT"] for r in resB])
    Q = unsort_cols(plan, [r["QT"] for r in resB])

    # ---- phase C
    P_b = bf16(P)
    ea_b = bf16(edge_attr)
    NCH = plan["NCH"]
    Q_b = bf16(Q)
    M2 = np.zeros((80, 64), np.float32)
    M2[0:64] = np.eye(64)
    M2[64:80] = C_.astype(np.float32)
    M2 = bf16(M2)
    if ("C", key, npos, bm2f) not in _CACHE:
        _CACHE[("C", key, npos, bm2f)] = build_phase_c(Cp, npos, bm2f)
    src_e, core_e = plan["src_e"], plan["core_e"]
    row, col = plan["row"], plan["col"]
    oe = plan["order_e"]
    mapsC = []
    for c in range(N_CORES):
        m = core_e == c
        G = np.zeros((80, NCH, 128), ml_dtypes.bfloat16)
        G[0:64, col[m], row[m]] = P_b[src_e[m]].T
        G[64:80, col[m], row[m]] = ea_b[oe[m]].T
        Qp = np.zeros((128, NPASS, 64), ml_dtypes.bfloat16)
        nr = plan["node_of_rank"][c].reshape(NPASS, 128)
        for p in range(NPASS):
            v = nr[p] >= 0
            Qp[v, p, :] = Q_b[nr[p][v]]
        mapsC.append(dict(G=G, Qp=Qp, M2=M2, ident=ident))
    LAST_RUNS.append((build_phase_c, dict(Cp=Cp, npos=npos, bm2=bm2f), mapsC))
    resC = _run(_CACHE[("C", key, npos, bm2f)], mapsC)

    out = np.empty(plan["E"], np.float32)
    for c in range(N_CORES):
        m = core_e == c
        out[oe[m]] = resC[c]["scores"][row[m], col[m]]
    return out
